# revision 1
# baseline (speedup 1.0000x reference)
"""ALiBi causal multi-head attention on 8 TRN2 NeuronCores.

Problem: x[2,2048,1024] -> qkv proj (16 heads, d=64) -> ALiBi-biased causal
softmax attention -> out proj [1024,1024] + bias.

Sharding: core = (batch b in {0,1}) x (head-group g in {0..3}, 4 heads each).
Each core computes its batch's QKV for its 4 heads, full causal attention,
and a partial output projection (its heads' rows of w_out). Host sums the 4
head-group partials per batch and adds b_out.

On-chip layout tricks:
  - x is pre-transposed on host (xT [1024, 2048] fp16), so the contraction
    dim is on partitions for all projection matmuls with no on-chip
    transposes.
  - sim is computed transposed (simT [keys, queries]) so exp(simT) is
    directly the lhsT of the attention*V matmul -- no P transposes.
  - The ALiBi bias slope*(j-i) is folded into the sim matmul via two extra
    contraction rows: kT_aug = [K^T; j; 1], qT_aug = [Q^T*scale; slope;
    -slope*i]. (The -slope*i fp16 rounding is constant per query column and
    cancels in softmax.)
  - The softmax denominator comes out of the attention*V matmul by
    appending a ones column to V (V_aug [keys, 65]); row 64 of the
    transposed output is sum_j P[j, i].
  - Causality: the strictly-masked key-tiles are skipped entirely; only
    diagonal 128x128 blocks get a -30000 additive mask before exp.
  - qT/kT are zero-padded to 96 partitions: K<=64 matmuls run at HALF rate
    on TRN2 (HW-measured: K=64 -> 427ns vs K=96 -> 216ns at N=512).
"""

import sys

for _p in ("/opt/trn_rl_repo", "/root/.axon_site/_ro/trn_rl_repo"):
    if _p not in sys.path:
        sys.path.append(_p)

import numpy as np
from math import log2, floor

import concourse.bass as bass
import concourse.mybir as mybir
import concourse.tile as tile
from concourse import bacc, bass_utils

F32 = mybir.dt.float32
F16 = mybir.dt.float16
AF = mybir.ActivationFunctionType

B = 2          # batches
NH = 16        # total heads
H = 4          # heads per core
D = 64         # head dim
N = 2048       # sequence length
DM = 1024      # model dim
CH = 512       # query chunk (free dim of sim/av matmuls)
NCH = N // CH  # 4
KT = N // 128  # 16 key tiles
KD = DM // 128 # 8 contraction tiles for projections
SCALE = D ** -0.5
MASK_NEG = -30000.0
N_WARMUP = 44  # dummy matmuls to warm the PE HAM clock during the DMA head


def _slopes(heads):
    def pow2_slopes(n):
        start = 2 ** (-(2 ** (-(log2(n) - 3))))
        return [start * (start ** i) for i in range(n)]
    if log2(heads).is_integer():
        return pow2_slopes(heads)
    c = 2 ** floor(log2(heads))
    return pow2_slopes(c) + pow2_slopes(2 * c)[0::2][: heads - c]


def build_program():
    nc = bacc.Bacc("TRN2", target_bir_lowering=False, debug=False, num_devices=8)
    xT = nc.dram_tensor("xT", [DM, N], F16, kind="ExternalInput").ap()
    wq = nc.dram_tensor("wq", [DM, H * D], F16, kind="ExternalInput").ap()
    wk = nc.dram_tensor("wk", [DM, H * D], F16, kind="ExternalInput").ap()
    wv = nc.dram_tensor("wv", [DM, H * D], F16, kind="ExternalInput").ap()
    wo = nc.dram_tensor("wo", [H * D, DM], F16, kind="ExternalInput").ap()
    qaug = nc.dram_tensor("qaug", [H, 2, N], F16, kind="ExternalInput").ap()
    kaug = nc.dram_tensor("kaug", [2, N], F16, kind="ExternalInput").ap()
    m0 = nc.dram_tensor("m0", [128, 128], F32, kind="ExternalInput").ap()
    out = nc.dram_tensor("out", [N, DM], F16, kind="ExternalOutput").ap()

    with tile.TileContext(nc) as tc:
        with tc.tile_pool(name="persist", bufs=1) as cp:
            # ---- constant / persistent tiles ----
            m0_sb = cp.tile([128, 128], F32, tag="m0", name="m0_sb")
            nc.sync.dma_start(m0_sb[:], m0[:])

            # DMA order matters: v-proj consumes wv+xt first, so issue those
            # interleaved; q/k weights next; everything else afterwards.
            xt, wqt, wkt, wvt = [], [], [], []
            for k in range(KD):
                t = cp.tile([128, H * D], F16, tag=f"wv{k}", name=f"wv{k}")
                nc.sync.dma_start(t[:], wv[128 * k:128 * (k + 1), :])
                wvt.append(t)
                t = cp.tile([128, N], F16, tag=f"xt{k}", name=f"xt{k}")
                nc.sync.dma_start(t[:], xT[128 * k:128 * (k + 1), :])
                xt.append(t)
            for nm, ap_, lst in (("wq", wq, wqt), ("wk", wk, wkt)):
                for k in range(KD):
                    t = cp.tile([128, H * D], F16, tag=f"{nm}{k}", name=f"{nm}{k}")
                    nc.sync.dma_start(t[:], ap_[128 * k:128 * (k + 1), :])
                    lst.append(t)

            qt, kt = [], []
            for h in range(H):
                tq = cp.tile([96, N], F16, tag=f"qt{h}", name=f"qt{h}")
                nc.gpsimd.memset(tq[64:96, :], 0.0)
                nc.sync.dma_start(tq[64:66, :], qaug[h])
                qt.append(tq)
                tk = cp.tile([96, N], F16, tag=f"kt{h}", name=f"kt{h}")
                nc.gpsimd.memset(tk[64:96, :], 0.0)
                nc.sync.dma_start(tk[64:66, :], kaug[:])
                kt.append(tk)

            wot = []
            for k in range(2):
                t = cp.tile([128, DM], F16, tag=f"wo{k}", name=f"wo{k}")
                nc.sync.dma_start(t[:], wo[128 * k:128 * (k + 1), :])
                wot.append(t)

            vsb = []
            for r in range(KT):
                t = cp.tile([128, 65 * H], F16, tag=f"v{r}", name=f"v{r}")
                for h in range(H):
                    nc.gpsimd.memset(t[:, 65 * h + 64:65 * h + 65], 1.0)
                vsb.append(t)

            avt = []
            for p in range(2):
                t = cp.tile([128, N], F16, tag=f"avt{p}", name=f"avt{p}")
                avt.append(t)

            # scratch operand for PE warm-up (contents irrelevant)
            warm = cp.tile([128, CH], F16, tag="warm", name="warm")
            nc.vector.memset(warm[0:128, 0:128], 0.0)

            # ---- pipelined by query chunk c:
            #   v-proj rows 4c..4c+3, q/k-proj chunk c, attention chunk c
            #   (all heads), out-proj rows 4c..4c+3
            with tc.tile_pool(name="psvqk", bufs=2, space="PSUM") as psqk, \
                 tc.tile_pool(name="psout", bufs=1, space="PSUM") as pso, \
                 tc.tile_pool(name="pssim", bufs=3, space="PSUM") as pss, \
                 tc.tile_pool(name="psav", bufs=2, space="PSUM") as psa, \
                 tc.tile_pool(name="ptp", bufs=6) as ptp, \
                 tc.tile_pool(name="smsb", bufs=3) as smsb, \
                 tc.tile_pool(name="osb", bufs=3) as osb:
                # PE warm-up: garbage matmuls with no input deps keep the PE
                # busy (and its HAM clock at 2.4 GHz) while the xT DMA lands.
                ps_w = pso.tile([128, CH], F32, tag="psout", name="ps_warm")
                for i in range(N_WARMUP):
                    nc.tensor.matmul(ps_w[:], warm[:, 0:128], warm[:],
                                     start=True, stop=True)

                for c in range(NCH):
                    # -- v rows for this chunk's new key tiles --
                    for r in range(4 * c, 4 * c + 4):
                        ps = psqk.tile([128, CH], F32, tag="vqk",
                                       name=f"psv{r}")
                        for k in range(KD):
                            nc.tensor.matmul(
                                ps[:, 0:H * D],
                                xt[k][:, 128 * r:128 * (r + 1)], wvt[k][:],
                                start=(k == 0), stop=(k == KD - 1))
                        for h in range(H):
                            nc.vector.tensor_copy(
                                vsb[r][:, 65 * h:65 * h + 64],
                                ps[:, 64 * h:64 * (h + 1)])
                    # -- qT / kT for this chunk (q/k interleaved so heads
                    #    0,1 are ready after two chains) --
                    for hp in range(H // 2):
                        for wt, dst in ((wqt, qt), (wkt, kt)):
                            ps = psqk.tile([128, CH], F32, tag="vqk",
                                           name=f"psqk{hp}_{c}")
                            for k in range(KD):
                                nc.tensor.matmul(
                                    ps[:],
                                    wt[k][:, 128 * hp:128 * (hp + 1)],
                                    xt[k][:, CH * c:CH * (c + 1)],
                                    start=(k == 0), stop=(k == KD - 1))
                            nc.vector.tensor_copy(
                                dst[2 * hp][0:64, CH * c:CH * (c + 1)],
                                ps[0:64, :])
                            nc.vector.tensor_copy(
                                dst[2 * hp + 1][0:64, CH * c:CH * (c + 1)],
                                ps[64:128, :])
                    # -- attention for this chunk, all heads --
                    nkt = 4 * c + 4
                    for h in range(H):
                        ps_av = psa.tile([65, CH], F32, tag="psav",
                                         name=f"psav{h}_{c}")
                        for t in range(nkt):
                            s = t - 4 * c
                            lo = 128 * s if s >= 0 else 0
                            ps_s = pss.tile([128, CH], F32, tag="pssim",
                                            name=f"pssim{h}_{c}_{t}")
                            nc.tensor.matmul(
                                ps_s[:, lo:CH],
                                kt[h][0:96, 128 * t:128 * (t + 1)],
                                qt[h][0:96, CH * c + lo:CH * (c + 1)],
                                start=True, stop=True)
                            if s >= 0:
                                nc.vector.tensor_add(
                                    ps_s[:, lo:lo + 128], ps_s[:, lo:lo + 128],
                                    m0_sb[:])
                            pt_t = ptp.tile([128, CH], F16, tag="pt",
                                            name=f"pt{h}_{c}_{t}")
                            nc.scalar.activation(
                                pt_t[:, lo:CH], ps_s[:, lo:CH], AF.Exp)
                            nc.tensor.matmul(
                                ps_av[:, lo:CH],
                                vsb[t][:, 65 * h:65 * h + 65],
                                pt_t[:, lo:CH],
                                start=(t == 0), stop=(t == nkt - 1))
                        # normalize: avt = ps_av[0:64] / denom (row 64)
                        dn32 = smsb.tile([1, CH], F32, tag="dn",
                                         name=f"dn{h}_{c}")
                        nc.scalar.activation(dn32[:], ps_av[64:65, :], AF.Copy)
                        rc32 = smsb.tile([1, CH], F32, tag="rc",
                                         name=f"rc{h}_{c}")
                        nc.vector.reciprocal_approx_fast(rc32[:], dn32[:])
                        rcb = smsb.tile([D, CH], F32, tag="rcb",
                                        name=f"rcb{h}_{c}")
                        nc.gpsimd.partition_broadcast(rcb[:], rc32[:])
                        nc.vector.tensor_mul(
                            avt[h // 2][64 * (h % 2):64 * (h % 2) + 64,
                                        CH * c:CH * (c + 1)],
                            ps_av[0:64, :], rcb[:])
                    # -- output projection for this chunk's query tiles --
                    for u in range(4 * c, 4 * c + 4):
                        o_sb = osb.tile([128, DM], F16, tag="osb",
                                        name=f"osb{u}")
                        for nchk in range(2):
                            ps = pso.tile([128, CH], F32, tag="psout",
                                          name=f"psout{u}_{nchk}")
                            for kk in range(2):
                                nc.tensor.matmul(
                                    ps[:],
                                    avt[kk][:, 128 * u:128 * (u + 1)],
                                    wot[kk][:, CH * nchk:CH * (nchk + 1)],
                                    start=(kk == 0), stop=(kk == 1))
                            # last chunk's copies run after all exps are
                            # done: put half on the (now idle) Scalar engine
                            # to halve the tail drain.
                            if c == NCH - 1 and nchk == 1:
                                nc.scalar.activation(
                                    o_sb[:, CH * nchk:CH * (nchk + 1)],
                                    ps[:], AF.Copy)
                            else:
                                nc.vector.tensor_copy(
                                    o_sb[:, CH * nchk:CH * (nchk + 1)], ps[:])
                        nc.sync.dma_start(out[128 * u:128 * (u + 1), :],
                                          o_sb[:])

    nc.compile()
    return nc


def make_in_maps(x, w_qkv, w_out):
    """Per-core numpy input dicts. Core c = batch (c // 4) x head-group (c % 4)."""
    slopes = _slopes(NH)
    pos = np.arange(N, dtype=np.float32)
    kaug = np.stack([pos.astype(np.float16),
                     np.ones(N, np.float16)])
    m0 = np.where(np.arange(128)[:, None] > np.arange(128)[None, :],
                  np.float32(MASK_NEG), np.float32(0.0))

    xT16 = [np.ascontiguousarray(x[b].T).astype(np.float16) for b in range(B)]

    in_maps = []
    for c in range(8):
        b, g = c // 4, c % 4
        hg0 = H * g
        cols = slice(hg0 * D, (hg0 + H) * D)
        wq = (w_qkv[:, cols.start:cols.stop] * SCALE).astype(np.float16)
        wk = w_qkv[:, DM + cols.start:DM + cols.stop].astype(np.float16)
        wv = w_qkv[:, 2 * DM + cols.start:2 * DM + cols.stop].astype(np.float16)
        wo = w_out[cols, :].astype(np.float16)
        qa = np.empty((H, 2, N), np.float16)
        for h in range(H):
            s16 = np.float16(slopes[hg0 + h])
            qa[h, 0, :] = s16
            qa[h, 1, :] = (-np.float32(s16) * pos).astype(np.float16)
        in_maps.append({
            "xT": xT16[b], "wq": wq, "wk": wk, "wv": wv, "wo": wo,
            "qaug": qa, "kaug": kaug, "m0": m0,
        })
    return in_maps


_NC_CACHE = []


def _get_nc():
    if not _NC_CACHE:
        _NC_CACHE.append(build_program())
    return _NC_CACHE[0]


def run_cores(in_maps, **kw):
    nc = _get_nc()
    return bass_utils.run_bass_kernel_spmd(nc, in_maps, core_ids=list(range(8)), **kw)


def kernel(x, w_qkv, w_out, b_out):
    x = np.asarray(x, np.float32)
    w_qkv = np.asarray(w_qkv, np.float32)
    w_out = np.asarray(w_out, np.float32)
    b_out = np.asarray(b_out, np.float32)
    res = run_cores(make_in_maps(x, w_qkv, w_out))
    out = np.zeros((B, N, DM), np.float32)
    for c in range(8):
        out[c // 4] += res.results[c]["out"].astype(np.float32)
    out += b_out[None, None, :]
    return out

